# revision 18
# baseline (speedup 1.0000x reference)
"""Bhattacharyya coefficient kernel for Trainium2 (8 NeuronCores, SPMD).

out[n,0,i,j] = (1/k^2) * sum_{c,p,q} w[c] * sqrt(x[n,c,i+p,j+q] * z[n,c,p,q])

Data-parallel over batch: 2 samples per core. Per sample:
  1. ACT: sx = sqrt(x) (fp8e4m3), szw = w/k^2 * sqrt(z) (fp8e4m3).
  2. TensorE fp8 DoubleRow: plane[t, y] = sum_c szw[c, t] * sx[c, y]
     for the 64 taps t = 8p+q and all 63*63 pixels y -- K=256 contracted
     in ONE matmul per <=512-column block (two 128-channel tiles per
     pass), M=64 taps.
  3. Evict PSUM -> SBUF (fp8, DVE), dump plane to DRAM scratch.
  4. Gather back with per-tap shifted offsets (flat DRAM AP):
     A[t, 441*r + u] = plane[t, 441*r + u + 63*(t>>3) + (t&7)] for 8
     out-row chunks r (7 output rows each), turning the tap-sum into a
     pure partition reduction.  Three scratch tensors (chunks 0-4 /
     5-6 / 7) keep each gather waiting only on the dumps it covers.
  5. TensorE one-hot matmuls: ps[r, u] = sum_t A[t, 441*r + u] via a
     [64, 8] fp8 stationary whose only nonzero column is r, accumulating
     all 8 chunks into one [8, 441] PSUM tile.  One DVE eviction
     compacts the valid 7x56 block per partition into obuf [8, 392];
     one DMA ships the contiguous [56, 56] output.

x arrives in decreasing-size pieces (1024x3/512/256/129 cols, both
channel halves per piece) with the two samples' pieces interleaved, so
both samples' serial tails (last piece -> sqrt -> matmul -> dump ->
gather -> reduce -> out) run concurrently right after the stream ends.
z is host-relayouted to [128, 2, 65] (channel-major, w packed in col
64) so it loads early with 520B descriptors.  All gathers ride HWDGE
rings (SWDGE completion is ~6us slower).
"""

import numpy as np

import concourse.bacc as bacc
import concourse.bass as bass
import concourse.mybir as mybir
from concourse import tile
from concourse.bass_utils import run_bass_kernel_spmd

N, C, KS, MS = 16, 256, 8, 63
MO = MS - KS + 1            # 56
F = MS * MS                 # 3969
NCORES = 8
SPC = N // NCORES           # samples per core
BLK = 512
AF = mybir.ActivationFunctionType
f32 = mybir.dt.float32
fp8 = mybir.dt.float8e4
DR = mybir.MatmulPerfMode.DoubleRow

RT = fp8                    # round-trip dtype for plane scratch

# x pieces (col_start, n_cols): decreasing sizes; last two split block 7
PIECES = [(0, 1024), (1024, 1024), (2048, 1024), (3072, 512),
          (3584, 256), (3840, 129)]
NBLOCKS = [(min(BLK, F - b * BLK)) for b in range(8)]   # 512 x7, 385

# stage-2: 8 out-row chunks of 7 rows; chunk r covers flat u in
# [441r, 441r+441), gathers plane cols [441r, 441r+889).
CW = 441
PIT_A = 5 * BLK             # scA: plane cols [0, 2560)      -> chunks 0-3
B_LO = 4 * CW               # 1764
PIT_B = 3584 - B_LO         # scB: plane cols [1764, 3584)   -> chunks 4-5
C_LO = 6 * CW               # 2646
PIT_C = 3976 - C_LO         # scC: plane cols [2646, 3976)   -> chunks 6-7

_CACHE = {}


def _build():
    nc = bacc.Bacc("TRN2", target_bir_lowering=False, debug=False)
    z_in = nc.declare_dram_parameter("zw", [SPC, 128, 2, 81], f32,
                                     isOutput=False)
    x_in = nc.declare_dram_parameter("x", [SPC, C, MS, MS], f32, isOutput=False)
    out = nc.declare_dram_parameter("out", [SPC, 1, MO, MO], f32, isOutput=True)

    scA = [nc.dram_tensor(f"pl_scA{s}", [64, PIT_A], RT) for s in range(SPC)]
    scB = [nc.dram_tensor(f"pl_scB{s}", [64, PIT_B], RT) for s in range(SPC)]
    scC = [nc.dram_tensor(f"pl_scC{s}", [64, PIT_C], RT) for s in range(SPC)]

    # [SPC, 128, 2, F]: partition = channel-within-half, then half, pixels
    xsrc = x_in.rearrange("s (k c) h w -> s c k (h w)", c=128)

    from contextlib import ExitStack

    with tile.TileContext(nc) as tc:
        with ExitStack() as stack:
            pool = lambda name, bufs, **kw: stack.enter_context(
                tc.tile_pool(name=name, bufs=bufs, **kw)
            )
            xpools = [pool(f"xq{i}", 2) for i in range(len(PIECES))]
            spools = [pool(f"sq{i}", 2) for i in range(len(PIECES))]
            zpool = pool("zpool", 8)
            plApool = pool("plApool", 2)
            pl7pool = pool("pl7pool", 2)
            g1pool = pool("g1pool", 8)
            ohpool = pool("ohpool", 8)
            obpool = pool("obpool", 2)
            psum = pool("psum", 4, space="PSUM")
            psum2 = pool("psum2", 2, space="PSUM")


            # x pieces on the Sync HWDGE ring, samples interleaved; zw
            # (host-relayouted, w folded into col 64, 520B rows) slots in
            # after the first two pieces -- early enough for szw, without
            # delaying the stream head
            xst = {}
            zts = []

            def load_x(s, pi):
                lo, ln = PIECES[pi]
                t = xpools[pi].tile([128, 2, ln], f32, tag=f"xst{pi}",
                                    name=f"x{s}p{pi}")
                nc.sync.dma_start(t[:], xsrc[s, :, :, lo : lo + ln])
                xst[(s, pi)] = t

            load_x(0, 0)
            load_x(1, 0)
            for s in range(SPC):
                zt = zpool.tile([128, 2, 81], f32, tag="zt", name=f"zt{s}")
                nc.sync.dma_start(zt[:], z_in[s])
                zts.append(zt)
            for pi in range(1, len(PIECES)):
                for s in range(SPC):
                    load_x(s, pi)

            # one-hot pair stationaries [128, 8] (rows 2t+i select output
            # row 2j+i), host-packed into zw cols 65:81 and cast to fp8
            ohs = []
            for j in range(4):
                oh = ohpool.tile([128, 8], RT, name=f"oh{j}")
                k, o = divmod(j, 2)
                nc.vector.tensor_copy(
                    oh[:], zts[0][:, k, 65 + 8 * o : 73 + 8 * o]
                )
                ohs.append(oh)

            # szw[c, k, t] = w[c]/64 * sqrt(z[c, t]); zw lands first so
            # these run before the x sqrts without stalling ACT
            szws = []
            for s in range(SPC):
                zsq = zpool.tile([128, 2, KS * KS], f32, tag="zsq",
                                 name=f"zsq{s}")
                szw = zpool.tile([128, 2, KS * KS], fp8, tag="szw",
                                 name=f"szw{s}")
                w64 = zpool.tile([128, 2], f32, tag="w64", name=f"w64_{s}")
                nc.vector.tensor_scalar_mul(
                    w64[:], zts[s][:, :, 64], 1.0 / (KS * KS)
                )
                for k in range(2):
                    nc.scalar.activation(
                        zsq[:, k, :], zts[s][:, k, 0:64], AF.Sqrt
                    )
                    nc.vector.tensor_scalar_mul(
                        szw[:, k, :], zsq[:, k, :], w64[:, k : k + 1]
                    )
                szws.append(szw)

            # ---- stage 1, piecewise, samples interleaved ----
            sxt = {}
            plAs, pl7s, ps7s = {}, {}, {}
            for s in range(SPC):
                plAs[s] = plApool.tile([64, 7 * BLK], RT, tag="plA",
                                       name=f"plA{s}")
                pl7s[s] = pl7pool.tile([64, 385], RT, tag="pl7", name=f"pl7{s}")

            def do_piece(s, pi):
                lo, ln = PIECES[pi]
                t = spools[pi].tile([128, 2, ln], fp8, tag=f"sxp{pi}",
                                    name=f"sx{s}p{pi}")
                nc.scalar.activation(t[:], xst[(s, pi)][:], AF.Sqrt)
                sxt[(s, pi)] = t
                szw = szws[s]
                plA, pl7 = plAs[s], pl7s[s]
                if pi < 4:
                    # whole 512-blocks lo/BLK .. (lo+ln)/BLK
                    for b in range(lo // BLK, (lo + ln) // BLK):
                        off = b * BLK - lo
                        ps = psum.tile([64, BLK], f32, tag="ps",
                                       name=f"ps{s}_{b}")
                        nc.tensor.matmul(
                            ps[:, :BLK], szw[:],
                            t[:, :, off : off + BLK],
                            start=True, stop=True, perf_mode=DR,
                        )
                        nc.vector.tensor_copy(
                            plA[:, b * BLK : (b + 1) * BLK], ps[:, :BLK]
                        )
                        eng = nc.scalar if s == 0 else nc.sync
                        if b == 4:
                            eng.dma_start(scA[s][:, :PIT_A], plA[:, :PIT_A])
                        elif b == 6:
                            eng.dma_start(scB[s][:], plA[:, B_LO:3584])
                            eng.dma_start(
                                scC[s][:, 0 : 3584 - C_LO], plA[:, C_LO:3584]
                            )
                else:
                    # block 7 sub-ranges (cols lo-3584 .. +ln within block 7)
                    o7 = lo - 7 * BLK
                    if pi == 4:
                        ps7s[s] = psum.tile([64, 385], f32, tag="ps",
                                            name=f"ps{s}_7")
                    ps = ps7s[s]
                    nc.tensor.matmul(
                        ps[:, o7 : o7 + ln], szw[:], t[:],
                        start=True, stop=True, perf_mode=DR,
                    )
                    if pi == 5:
                        nc.vector.tensor_copy(pl7[:, :385], ps[:, :385])
                        eng = nc.scalar if s == 0 else nc.sync
                        eng.dma_start(
                            scC[s][:, 3584 - C_LO : 3584 - C_LO + 385],
                            pl7[:, :385],
                        )

            for pi in range(len(PIECES)):
                for s in range(SPC):
                    do_piece(s, pi)

            # ---- stage 2 ----
            def gathers(s, eng):
                gps = []
                for j, (sc, pit, base) in enumerate([
                    (scA, PIT_A, 0), (scA, PIT_A, 2 * CW),
                    (scB, PIT_B, 0), (scC, PIT_C, 0),
                ]):
                    g = g1pool.tile([128, CW], RT, tag=f"gp{j}{s}",
                                    name=f"g{j}_{s}")
                    for r in range(2):
                        eng.dma_start(g[64 * r : 64 * r + 64, :], bass.AP(
                            sc[s][:].tensor, base + r * CW,
                            [[8 * pit + MS, 8], [pit + 1, 8], [1, CW]],
                        ))
                    gps.append(g)
                return gps

            gts = [gathers(0, nc.scalar), gathers(1, nc.sync)]

            for s in range(SPC):
                ps2 = psum2.tile([8, CW], f32, tag="ps2", name=f"ps2_{s}")
                for j in range(4):
                    nc.tensor.matmul(
                        ps2[:, :CW],
                        ohs[j][:],
                        gts[s][j][:],
                        start=(j == 0),
                        stop=(j == 3),
                    )

                obuf = obpool.tile([8, 7 * MO], f32, tag="ob", name=f"ob{s}")
                psv = ps2[:].rearrange("p (i j) -> p i j", j=MS)[:, :, 0:MO]
                nc.vector.tensor_copy(
                    obuf[:].rearrange("p (i j) -> p i j", j=MO), psv
                )
                nc.scalar.dma_start(
                    out[s].rearrange("c (r i) j -> (c r) (i j)", r=8), obuf[:]
                )

    nc.compile()
    return nc


def _get_nc():
    if "nc" not in _CACHE:
        _CACHE["nc"] = _build()
    return _CACHE["nc"]


def _run(z, x, weights, **runkw):
    z = np.asarray(z, dtype=np.float32)
    x = np.ascontiguousarray(np.asarray(x), dtype=np.float32)
    w = np.asarray(weights, dtype=np.float32).reshape(C)
    # host relayout: zw[n, c, k, 0:64] = z[n, 128k+c, p, q]; col 64 = w;
    # cols 65:81 = interleaved one-hot pair patterns for stage 2
    zw = np.empty((N, 128, 2, 81), dtype=np.float32)
    zw[:, :, :, 0:64] = z.reshape(N, 2, 128, KS * KS).transpose(0, 2, 1, 3)
    zw[:, :, :, 64] = w.reshape(2, 128).T[None]
    ohm = np.zeros((4, 128, 8), dtype=np.float32)
    for j in range(4):
        ohm[j, 0::2, 2 * j] = 1.0
        ohm[j, 1::2, 2 * j + 1] = 1.0
    for j in range(4):
        k, o = divmod(j, 2)
        zw[:, :, k, 65 + 8 * o : 73 + 8 * o] = ohm[j][None]
    zw = np.ascontiguousarray(zw)
    in_maps = []
    for i in range(NCORES):
        lo, hi = i * SPC, (i + 1) * SPC
        in_maps.append({"zw": zw[lo:hi], "x": x[lo:hi]})
    nc = _get_nc()
    try:
        res = run_bass_kernel_spmd(
            nc, in_maps, core_ids=list(range(NCORES)), **runkw
        )
    except Exception:
        # transient device errors (e.g. NRT exec-unit unrecoverable) have
        # been observed to succeed on retry
        res = run_bass_kernel_spmd(
            nc, in_maps, core_ids=list(range(NCORES)), **runkw
        )
    full = np.concatenate([res.results[i]["out"] for i in range(NCORES)], axis=0)
    return full, res


def kernel(z, x, weights):
    full, _ = _run(z, x, weights)
    return full


# revision 19
# speedup vs baseline: 1.1267x; 1.1267x over previous
"""Bhattacharyya coefficient kernel for Trainium2 (8 NeuronCores, SPMD).

out[n,0,i,j] = (1/k^2) * sum_{c,p,q} w[c] * sqrt(x[n,c,i+p,j+q] * z[n,c,p,q])

Data-parallel over batch: 2 samples per core. Per sample:
  1. ACT: sx = sqrt(x) (fp8e4m3), szw = w/k^2 * sqrt(z) (fp8e4m3).
  2. TensorE fp8 DoubleRow: plane[t, y] = sum_c szw[c, t] * sx[c, y]
     for the 64 taps t = 8p+q and all 63*63 pixels y -- K=256 contracted
     in ONE matmul per <=512-column block (two 128-channel tiles per
     pass), M=64 taps.
  3. Evict PSUM -> SBUF (fp8, DVE), dump plane to DRAM scratch.
  4. Gather back with per-tap shifted offsets (flat DRAM AP):
     A[t, 441*r + u] = plane[t, 441*r + u + 63*(t>>3) + (t&7)] for 8
     out-row chunks r (7 output rows each), turning the tap-sum into a
     pure partition reduction.  Three scratch tensors (chunks 0-4 /
     5-6 / 7) keep each gather waiting only on the dumps it covers.
  5. TensorE one-hot matmuls: ps[r, u] = sum_t A[t, 441*r + u] via a
     [64, 8] fp8 stationary whose only nonzero column is r, accumulating
     all 8 chunks into one [8, 441] PSUM tile.  One DVE eviction
     compacts the valid 7x56 block per partition into obuf [8, 392];
     one DMA ships the contiguous [56, 56] output.

x arrives in decreasing-size pieces (1024x3/512/256/129 cols, both
channel halves per piece) with the two samples' pieces interleaved, so
both samples' serial tails (last piece -> sqrt -> matmul -> dump ->
gather -> reduce -> out) run concurrently right after the stream ends.
z is host-relayouted to [128, 2, 65] (channel-major, w packed in col
64) so it loads early with 520B descriptors.  All gathers ride HWDGE
rings (SWDGE completion is ~6us slower).
"""

import numpy as np

import concourse.bacc as bacc
import concourse.bass as bass
import concourse.mybir as mybir
from concourse import tile
from concourse.bass_utils import run_bass_kernel_spmd

N, C, KS, MS = 16, 256, 8, 63
MO = MS - KS + 1            # 56
F = MS * MS                 # 3969
NCORES = 8
SPC = N // NCORES           # samples per core
BLK = 512
AF = mybir.ActivationFunctionType
f32 = mybir.dt.float32
fp8 = mybir.dt.float8e4
DR = mybir.MatmulPerfMode.DoubleRow

RT = fp8                    # round-trip dtype for plane scratch

# x pieces (col_start, n_cols): decreasing sizes; last two split block 7
PIECES = [(0, 1024), (1024, 1024), (2048, 1024), (3072, 512),
          (3584, 256), (3840, 129)]
NBLOCKS = [(min(BLK, F - b * BLK)) for b in range(8)]   # 512 x7, 385

# stage-2: 8 out-row chunks of 7 rows; chunk r covers flat u in
# [441r, 441r+441), gathers plane cols [441r, 441r+889).
CW = 441
PIT_A = 5 * BLK             # scA: plane cols [0, 2560)      -> chunks 0-3
B_LO = 4 * CW               # 1764
PIT_B = 3584 - B_LO         # scB: plane cols [1764, 3584)   -> chunks 4-5
C_LO = 6 * CW               # 2646
PIT_C = 3976 - C_LO         # scC: plane cols [2646, 3976)   -> chunks 6-7

_CACHE = {}


def _build():
    nc = bacc.Bacc("TRN2", target_bir_lowering=False, debug=False)
    z_in = nc.declare_dram_parameter("zw", [SPC, 128, 2, 81], f32,
                                     isOutput=False)
    x_in = nc.declare_dram_parameter("x", [SPC, C, MS, MS], f32, isOutput=False)
    out = nc.declare_dram_parameter("out", [SPC, 1, MO, MO], f32, isOutput=True)

    scA = [nc.dram_tensor(f"pl_scA{s}", [64, PIT_A], RT) for s in range(SPC)]
    scB = [nc.dram_tensor(f"pl_scB{s}", [64, PIT_B], RT) for s in range(SPC)]
    scC = [nc.dram_tensor(f"pl_scC{s}", [64, PIT_C], RT) for s in range(SPC)]

    # [SPC, 128, 2, F]: partition = channel-within-half, then half, pixels
    xsrc = x_in.rearrange("s (k c) h w -> s c k (h w)", c=128)

    from contextlib import ExitStack

    with tile.TileContext(nc) as tc:
        with ExitStack() as stack:
            pool = lambda name, bufs, **kw: stack.enter_context(
                tc.tile_pool(name=name, bufs=bufs, **kw)
            )
            xpools = [pool(f"xq{i}", 2) for i in range(len(PIECES))]
            spools = [pool(f"sq{i}", 2) for i in range(len(PIECES))]
            zpool = pool("zpool", 8)
            plApool = pool("plApool", 2)
            pl7pool = pool("pl7pool", 2)
            g1pool = pool("g1pool", 8)
            ohpool = pool("ohpool", 8)
            obpool = pool("obpool", 2)
            psum = pool("psum", 4, space="PSUM")
            psum2 = pool("psum2", 2, space="PSUM")


            # x pieces on the Sync HWDGE ring, samples interleaved; zw
            # (host-relayouted, w folded into col 64, 520B rows) slots in
            # after the first two pieces -- early enough for szw, without
            # delaying the stream head
            xst = {}
            zts = []

            def load_x(s, pi):
                lo, ln = PIECES[pi]
                t = xpools[pi].tile([128, 2, ln], f32, tag=f"xst{pi}",
                                    name=f"x{s}p{pi}")
                nc.sync.dma_start(t[:], xsrc[s, :, :, lo : lo + ln])
                xst[(s, pi)] = t

            load_x(0, 0)
            load_x(1, 0)
            for s in range(SPC):
                zt = zpool.tile([128, 2, 81], f32, tag="zt", name=f"zt{s}")
                nc.sync.dma_start(zt[:], z_in[s])
                zts.append(zt)
            for pi in range(1, len(PIECES)):
                for s in range(SPC):
                    load_x(s, pi)

            # one-hot [64, 8] stationaries for the chunked tap reduction
            ohs = []
            for r in range(8):
                oh = ohpool.tile([64, 8], RT, name=f"oh{r}")
                nc.gpsimd.memset(oh[:], 0.0)
                nc.gpsimd.memset(oh[:, r : r + 1], 1.0)
                ohs.append(oh)

            # szw[c, k, t] = w[c]/64 * sqrt(z[c, t]); zw lands first so
            # these run before the x sqrts without stalling ACT
            szws = []
            for s in range(SPC):
                zsq = zpool.tile([128, 2, KS * KS], f32, tag="zsq",
                                 name=f"zsq{s}")
                szw = zpool.tile([128, 2, KS * KS], fp8, tag="szw",
                                 name=f"szw{s}")
                w64 = zpool.tile([128, 2], f32, tag="w64", name=f"w64_{s}")
                nc.vector.tensor_scalar_mul(
                    w64[:], zts[s][:, :, 64], 1.0 / (KS * KS)
                )
                for k in range(2):
                    nc.scalar.activation(
                        zsq[:, k, :], zts[s][:, k, 0:64], AF.Sqrt
                    )
                    nc.vector.tensor_scalar_mul(
                        szw[:, k, :], zsq[:, k, :], w64[:, k : k + 1]
                    )
                szws.append(szw)

            # ---- stage 1, piecewise, samples interleaved ----
            sxt = {}
            plAs, pl7s, ps7s = {}, {}, {}
            for s in range(SPC):
                plAs[s] = plApool.tile([64, 7 * BLK], RT, tag="plA",
                                       name=f"plA{s}")
                pl7s[s] = pl7pool.tile([64, 385], RT, tag="pl7", name=f"pl7{s}")

            def do_piece(s, pi):
                lo, ln = PIECES[pi]
                t = spools[pi].tile([128, 2, ln], fp8, tag=f"sxp{pi}",
                                    name=f"sx{s}p{pi}")
                nc.scalar.activation(t[:], xst[(s, pi)][:], AF.Sqrt)
                sxt[(s, pi)] = t
                szw = szws[s]
                plA, pl7 = plAs[s], pl7s[s]
                if pi < 4:
                    # whole 512-blocks lo/BLK .. (lo+ln)/BLK
                    for b in range(lo // BLK, (lo + ln) // BLK):
                        off = b * BLK - lo
                        ps = psum.tile([64, BLK], f32, tag="ps",
                                       name=f"ps{s}_{b}")
                        nc.tensor.matmul(
                            ps[:, :BLK], szw[:],
                            t[:, :, off : off + BLK],
                            start=True, stop=True, perf_mode=DR,
                        )
                        nc.vector.tensor_copy(
                            plA[:, b * BLK : (b + 1) * BLK], ps[:, :BLK]
                        )
                        eng = nc.scalar if s == 0 else nc.sync
                        if b == 4:
                            eng.dma_start(scA[s][:, :PIT_A], plA[:, :PIT_A])
                        elif b == 6:
                            eng.dma_start(scB[s][:], plA[:, B_LO:3584])
                            eng.dma_start(
                                scC[s][:, 0 : 3584 - C_LO], plA[:, C_LO:3584]
                            )
                else:
                    # block 7 sub-ranges (cols lo-3584 .. +ln within block 7)
                    o7 = lo - 7 * BLK
                    if pi == 4:
                        ps7s[s] = psum.tile([64, 385], f32, tag="ps",
                                            name=f"ps{s}_7")
                    ps = ps7s[s]
                    nc.tensor.matmul(
                        ps[:, o7 : o7 + ln], szw[:], t[:],
                        start=True, stop=True, perf_mode=DR,
                    )
                    if pi == 5:
                        nc.vector.tensor_copy(pl7[:, :385], ps[:, :385])
                        eng = nc.scalar if s == 0 else nc.sync
                        eng.dma_start(
                            scC[s][:, 3584 - C_LO : 3584 - C_LO + 385],
                            pl7[:, :385],
                        )

            for pi in range(len(PIECES)):
                for s in range(SPC):
                    do_piece(s, pi)

            # ---- stage 2 ----
            def gathers(s, eng):
                gps = []
                for j, (sc, pit, base) in enumerate([
                    (scA, PIT_A, 0), (scA, PIT_A, 2 * CW),
                    (scB, PIT_B, 0), (scC, PIT_C, 0),
                ]):
                    g = g1pool.tile([64, 2, CW], RT, tag=f"gp{j}{s}",
                                    name=f"g{j}_{s}")
                    eng.dma_start(g[:], bass.AP(
                        sc[s][:].tensor, base,
                        [[8 * pit + MS, 8], [pit + 1, 8], [CW, 2], [1, CW]],
                    ))
                    gps.append(g)
                return gps

            gts = [gathers(0, nc.scalar), gathers(1, nc.sync)]

            for s in range(SPC):
                ps2 = psum2.tile([8, CW], f32, tag="ps2", name=f"ps2_{s}")
                for r in range(8):
                    nc.tensor.matmul(
                        ps2[:, :CW],
                        ohs[r][:],
                        gts[s][r // 2][:, r % 2, :],
                        start=(r == 0),
                        stop=(r == 7),
                    )

                obuf = obpool.tile([8, 7 * MO], f32, tag="ob", name=f"ob{s}")
                psv = ps2[:].rearrange("p (i j) -> p i j", j=MS)[:, :, 0:MO]
                nc.vector.tensor_copy(
                    obuf[:].rearrange("p (i j) -> p i j", j=MO), psv
                )
                nc.scalar.dma_start(
                    out[s].rearrange("c (r i) j -> (c r) (i j)", r=8), obuf[:]
                )

    nc.compile()
    return nc


def _get_nc():
    if "nc" not in _CACHE:
        _CACHE["nc"] = _build()
    return _CACHE["nc"]


def _run(z, x, weights, **runkw):
    z = np.asarray(z, dtype=np.float32)
    x = np.ascontiguousarray(np.asarray(x), dtype=np.float32)
    w = np.asarray(weights, dtype=np.float32).reshape(C)
    # host relayout: zw[n, c, k, 0:64] = z[n, 128k+c, p, q]; col 64 = w;
    # cols 65:81 = interleaved one-hot pair patterns for stage 2
    zw = np.empty((N, 128, 2, 81), dtype=np.float32)
    zw[:, :, :, 0:64] = z.reshape(N, 2, 128, KS * KS).transpose(0, 2, 1, 3)
    zw[:, :, :, 64] = w.reshape(2, 128).T[None]
    ohm = np.zeros((4, 128, 8), dtype=np.float32)
    for j in range(4):
        ohm[j, 0::2, 2 * j] = 1.0
        ohm[j, 1::2, 2 * j + 1] = 1.0
    for j in range(4):
        k, o = divmod(j, 2)
        zw[:, :, k, 65 + 8 * o : 73 + 8 * o] = ohm[j][None]
    zw = np.ascontiguousarray(zw)
    in_maps = []
    for i in range(NCORES):
        lo, hi = i * SPC, (i + 1) * SPC
        in_maps.append({"zw": zw[lo:hi], "x": x[lo:hi]})
    nc = _get_nc()
    try:
        res = run_bass_kernel_spmd(
            nc, in_maps, core_ids=list(range(NCORES)), **runkw
        )
    except Exception:
        # transient device errors (e.g. NRT exec-unit unrecoverable) have
        # been observed to succeed on retry
        res = run_bass_kernel_spmd(
            nc, in_maps, core_ids=list(range(NCORES)), **runkw
        )
    full = np.concatenate([res.results[i]["out"] for i in range(NCORES)], axis=0)
    return full, res


def kernel(z, x, weights):
    full, _ = _run(z, x, weights)
    return full


# revision 20
# speedup vs baseline: 1.1504x; 1.0210x over previous
"""Bhattacharyya coefficient kernel for Trainium2 (8 NeuronCores, SPMD).

out[n,0,i,j] = (1/k^2) * sum_{c,p,q} w[c] * sqrt(x[n,c,i+p,j+q] * z[n,c,p,q])

Data-parallel over batch: 2 samples per core. Per sample:
  1. ACT: sx = sqrt(x) (fp8e4m3), szw = w/k^2 * sqrt(z) (fp8e4m3).
  2. TensorE fp8 DoubleRow: plane[t, y] = sum_c szw[c, t] * sx[c, y]
     for the 64 taps t = 8p+q and all 63*63 pixels y -- K=256 contracted
     in ONE matmul per <=512-column block (two 128-channel tiles per
     pass), M=64 taps.
  3. Evict PSUM -> SBUF (fp8, DVE), dump plane to DRAM scratch.
  4. Gather back with per-tap shifted offsets (flat DRAM AP):
     A[t, 441*r + u] = plane[t, 441*r + u + 63*(t>>3) + (t&7)] for 8
     out-row chunks r (7 output rows each), turning the tap-sum into a
     pure partition reduction.  Three scratch tensors (chunks 0-4 /
     5-6 / 7) keep each gather waiting only on the dumps it covers.
  5. TensorE one-hot matmuls: ps[r, u] = sum_t A[t, 441*r + u] via a
     [64, 8] fp8 stationary whose only nonzero column is r, accumulating
     all 8 chunks into one [8, 441] PSUM tile.  One DVE eviction
     compacts the valid 7x56 block per partition into obuf [8, 392];
     one DMA ships the contiguous [56, 56] output.

x arrives in decreasing-size pieces (1024x3/512/256/129 cols, both
channel halves per piece) with the two samples' pieces interleaved, so
both samples' serial tails (last piece -> sqrt -> matmul -> dump ->
gather -> reduce -> out) run concurrently right after the stream ends.
z is host-relayouted to [128, 2, 65] (channel-major, w packed in col
64) so it loads early with 520B descriptors.  All gathers ride HWDGE
rings (SWDGE completion is ~6us slower).
"""

import numpy as np

import concourse.bacc as bacc
import concourse.bass as bass
import concourse.mybir as mybir
from concourse import tile
from concourse.bass_utils import run_bass_kernel_spmd

N, C, KS, MS = 16, 256, 8, 63
MO = MS - KS + 1            # 56
F = MS * MS                 # 3969
NCORES = 8
SPC = N // NCORES           # samples per core
BLK = 512
AF = mybir.ActivationFunctionType
f32 = mybir.dt.float32
fp8 = mybir.dt.float8e4
DR = mybir.MatmulPerfMode.DoubleRow

RT = fp8                    # round-trip dtype for plane scratch

# x pieces (col_start, n_cols): decreasing sizes; last two split block 7
PIECES = [(0, 1024), (1024, 1024), (2048, 1024), (3072, 512),
          (3584, 256), (3840, 129)]
NBLOCKS = [(min(BLK, F - b * BLK)) for b in range(8)]   # 512 x7, 385

# stage-2: 8 out-row chunks of 7 rows; chunk r covers flat u in
# [441r, 441r+441), gathers plane cols [441r, 441r+889).
CW = 441
PIT_A = 2212                # scA: plane cols [0, 2212)      -> chunks 0-3
B_LO = 4 * CW               # 1764
PIT_B = 1332                # scB: plane cols [1764, 3096)   -> chunks 4-5
C_LO = 6 * CW               # 2646
PIT_C = 3976 - C_LO         # scC: plane cols [2646, 3976)   -> chunks 6-7

_CACHE = {}


def _build():
    nc = bacc.Bacc("TRN2", target_bir_lowering=False, debug=False)
    z_in = nc.declare_dram_parameter("zw", [SPC, 128, 2, 81], f32,
                                     isOutput=False)
    x_in = nc.declare_dram_parameter("x", [SPC, C, MS, MS], f32, isOutput=False)
    out = nc.declare_dram_parameter("out", [SPC, 1, MO, MO], f32, isOutput=True)

    scA = [nc.dram_tensor(f"pl_scA{s}", [64, PIT_A], RT) for s in range(SPC)]
    scB = [nc.dram_tensor(f"pl_scB{s}", [64, PIT_B], RT) for s in range(SPC)]
    scC = [nc.dram_tensor(f"pl_scC{s}", [64, PIT_C], RT) for s in range(SPC)]

    # [SPC, 128, 2, F]: partition = channel-within-half, then half, pixels
    xsrc = x_in.rearrange("s (k c) h w -> s c k (h w)", c=128)

    from contextlib import ExitStack

    with tile.TileContext(nc) as tc:
        with ExitStack() as stack:
            pool = lambda name, bufs, **kw: stack.enter_context(
                tc.tile_pool(name=name, bufs=bufs, **kw)
            )
            xpools = [pool(f"xq{i}", 2) for i in range(len(PIECES))]
            spools = [pool(f"sq{i}", 2) for i in range(len(PIECES))]
            zpool = pool("zpool", 8)
            plApool = pool("plApool", 2)
            pl7pool = pool("pl7pool", 2)
            g1pool = pool("g1pool", 8)
            ohpool = pool("ohpool", 8)
            obpool = pool("obpool", 2)
            psum = pool("psum", 4, space="PSUM")
            psum2 = pool("psum2", 2, space="PSUM")


            # x pieces on the Sync HWDGE ring, samples interleaved; zw
            # (host-relayouted, w folded into col 64, 520B rows) slots in
            # after the first two pieces -- early enough for szw, without
            # delaying the stream head
            xst = {}
            zts = []

            def load_x(s, pi):
                lo, ln = PIECES[pi]
                t = xpools[pi].tile([128, 2, ln], f32, tag=f"xst{pi}",
                                    name=f"x{s}p{pi}")
                nc.sync.dma_start(t[:], xsrc[s, :, :, lo : lo + ln])
                xst[(s, pi)] = t

            load_x(0, 0)
            load_x(1, 0)
            for s in range(SPC):
                zt = zpool.tile([128, 2, 81], f32, tag="zt", name=f"zt{s}")
                nc.sync.dma_start(zt[:], z_in[s])
                zts.append(zt)
            for pi in range(1, len(PIECES)):
                for s in range(SPC):
                    load_x(s, pi)

            # one-hot [64, 8] stationaries for the chunked tap reduction
            ohs = []
            for r in range(8):
                oh = ohpool.tile([64, 8], RT, name=f"oh{r}")
                nc.gpsimd.memset(oh[:], 0.0)
                nc.gpsimd.memset(oh[:, r : r + 1], 1.0)
                ohs.append(oh)

            # szw[c, k, t] = w[c]/64 * sqrt(z[c, t]); zw lands first so
            # these run before the x sqrts without stalling ACT
            szws = []
            for s in range(SPC):
                zsq = zpool.tile([128, 2, KS * KS], f32, tag="zsq",
                                 name=f"zsq{s}")
                szw = zpool.tile([128, 2, KS * KS], fp8, tag="szw",
                                 name=f"szw{s}")
                w64 = zpool.tile([128, 2], f32, tag="w64", name=f"w64_{s}")
                nc.vector.tensor_scalar_mul(
                    w64[:], zts[s][:, :, 64], 1.0 / (KS * KS)
                )
                for k in range(2):
                    nc.scalar.activation(
                        zsq[:, k, :], zts[s][:, k, 0:64], AF.Sqrt
                    )
                    nc.vector.tensor_scalar_mul(
                        szw[:, k, :], zsq[:, k, :], w64[:, k : k + 1]
                    )
                szws.append(szw)

            # ---- stage 1, piecewise, samples interleaved ----
            sxt = {}
            plAs, pl7s, ps7s = {}, {}, {}
            for s in range(SPC):
                plAs[s] = plApool.tile([64, 7 * BLK], RT, tag="plA",
                                       name=f"plA{s}")
                pl7s[s] = pl7pool.tile([64, 385], RT, tag="pl7", name=f"pl7{s}")

            def do_piece(s, pi):
                lo, ln = PIECES[pi]
                t = spools[pi].tile([128, 2, ln], fp8, tag=f"sxp{pi}",
                                    name=f"sx{s}p{pi}")
                nc.scalar.activation(t[:], xst[(s, pi)][:], AF.Sqrt)
                sxt[(s, pi)] = t
                szw = szws[s]
                plA, pl7 = plAs[s], pl7s[s]
                if pi < 4:
                    # whole 512-blocks lo/BLK .. (lo+ln)/BLK
                    for b in range(lo // BLK, (lo + ln) // BLK):
                        off = b * BLK - lo
                        ps = psum.tile([64, BLK], f32, tag="ps",
                                       name=f"ps{s}_{b}")
                        nc.tensor.matmul(
                            ps[:, :BLK], szw[:],
                            t[:, :, off : off + BLK],
                            start=True, stop=True, perf_mode=DR,
                        )
                        nc.vector.tensor_copy(
                            plA[:, b * BLK : (b + 1) * BLK], ps[:, :BLK]
                        )
                        eng = nc.scalar if s == 0 else nc.sync
                        if b == 5:
                            eng.dma_start(scA[s][:, :PIT_A], plA[:, :PIT_A])
                        elif b == 6:
                            eng.dma_start(scB[s][:],
                                          plA[:, B_LO : B_LO + PIT_B])
                            eng.dma_start(
                                scC[s][:, 0 : 3584 - C_LO], plA[:, C_LO:3584]
                            )
                else:
                    # block 7 sub-ranges (cols lo-3584 .. +ln within block 7)
                    o7 = lo - 7 * BLK
                    if pi == 4:
                        ps7s[s] = psum.tile([64, 385], f32, tag="ps",
                                            name=f"ps{s}_7")
                    ps = ps7s[s]
                    nc.tensor.matmul(
                        ps[:, o7 : o7 + ln], szw[:], t[:],
                        start=True, stop=True, perf_mode=DR,
                    )
                    if pi == 5:
                        nc.vector.tensor_copy(pl7[:, :385], ps[:, :385])
                        eng = nc.scalar if s == 0 else nc.sync
                        eng.dma_start(
                            scC[s][:, 3584 - C_LO : 3584 - C_LO + 385],
                            pl7[:, :385],
                        )

            for pi in range(len(PIECES)):
                for s in range(SPC):
                    do_piece(s, pi)

            # ---- stage 2 ----
            def gathers(s, eng):
                gps = []
                for j, (sc, pit, base) in enumerate([
                    (scA, PIT_A, 0), (scA, PIT_A, 2 * CW),
                    (scB, PIT_B, 0), (scC, PIT_C, 0),
                ]):
                    g = g1pool.tile([64, 2, CW], RT, tag=f"gp{j}{s}",
                                    name=f"g{j}_{s}")
                    eng.dma_start(g[:], bass.AP(
                        sc[s][:].tensor, base,
                        [[8 * pit + MS, 8], [pit + 1, 8], [CW, 2], [1, CW]],
                    ))
                    gps.append(g)
                return gps

            gts = [gathers(0, nc.scalar), gathers(1, nc.sync)]

            for s in range(SPC):
                ps2 = psum2.tile([8, CW], f32, tag="ps2", name=f"ps2_{s}")
                for r in range(8):
                    nc.tensor.matmul(
                        ps2[:, :CW],
                        ohs[r][:],
                        gts[s][r // 2][:, r % 2, :],
                        start=(r == 0),
                        stop=(r == 7),
                    )

                obuf = obpool.tile([8, 7 * MO], f32, tag="ob", name=f"ob{s}")
                psv = ps2[:].rearrange("p (i j) -> p i j", j=MS)[:, :, 0:MO]
                nc.vector.tensor_copy(
                    obuf[:].rearrange("p (i j) -> p i j", j=MO), psv
                )
                nc.scalar.dma_start(
                    out[s].rearrange("c (r i) j -> (c r) (i j)", r=8), obuf[:]
                )

    nc.compile()
    return nc


def _get_nc():
    if "nc" not in _CACHE:
        _CACHE["nc"] = _build()
    return _CACHE["nc"]


def _run(z, x, weights, **runkw):
    z = np.asarray(z, dtype=np.float32)
    x = np.ascontiguousarray(np.asarray(x), dtype=np.float32)
    w = np.asarray(weights, dtype=np.float32).reshape(C)
    # host relayout: zw[n, c, k, 0:64] = z[n, 128k+c, p, q]; col 64 = w;
    # cols 65:81 = interleaved one-hot pair patterns for stage 2
    zw = np.empty((N, 128, 2, 81), dtype=np.float32)
    zw[:, :, :, 0:64] = z.reshape(N, 2, 128, KS * KS).transpose(0, 2, 1, 3)
    zw[:, :, :, 64] = w.reshape(2, 128).T[None]
    ohm = np.zeros((4, 128, 8), dtype=np.float32)
    for j in range(4):
        ohm[j, 0::2, 2 * j] = 1.0
        ohm[j, 1::2, 2 * j + 1] = 1.0
    for j in range(4):
        k, o = divmod(j, 2)
        zw[:, :, k, 65 + 8 * o : 73 + 8 * o] = ohm[j][None]
    zw = np.ascontiguousarray(zw)
    in_maps = []
    for i in range(NCORES):
        lo, hi = i * SPC, (i + 1) * SPC
        in_maps.append({"zw": zw[lo:hi], "x": x[lo:hi]})
    nc = _get_nc()
    try:
        res = run_bass_kernel_spmd(
            nc, in_maps, core_ids=list(range(NCORES)), **runkw
        )
    except Exception:
        # transient device errors (e.g. NRT exec-unit unrecoverable) have
        # been observed to succeed on retry
        res = run_bass_kernel_spmd(
            nc, in_maps, core_ids=list(range(NCORES)), **runkw
        )
    full = np.concatenate([res.results[i]["out"] for i in range(NCORES)], axis=0)
    return full, res


def kernel(z, x, weights):
    full, _ = _run(z, x, weights)
    return full
